# revision 53
# baseline (speedup 1.0000x reference)
"""Trainium2 Bass kernel for nn_CrossAttention_249108103802.

8 cores data-parallel over B=8; per core (batch b):
  G_s   = x_s^T x_s            (Gram, fp16 operands, fp32 psum, upper tri)
  A_s   = (G_s - mu I) Wv_s    (fp16)
  ctp_s = A_s^T Wk_s + mu Wv_s^T Wk_s   (fp16 pair-packed + fp32 TT)
  ctx_s = softmax_d(scale * ctp_s)      (per-head 64x64)
  o2^T  = blockdiag(ctx1) stationary @ xT2   fused into x2 streaming
  o1^T  = same with ctx2 / resident xT1

Everything is tuned against the cost model: fp16 operands (1 col/cycle,
half the DMA), PE kept gap-free (p-state halves the clock for 3us after
any idle), a single serialized DMA pipe (issue order == transfer order).
xT1 is built with PE transposes during phase 1 (PE has slack there);
xT2 arrives via DMA-transpose loads (pipe has slack, and phase 2's
copy engines are freed to evacuate the fused o2 matmuls). The softmax
tails are interleaved with matmul filler (phase-2 gram tiles for
tail 1; deferred o2 + pipelined o1 head-pair streams for tail 2).
Outputs are written as o^T [C, N] fp16 and transposed back on host.
"""
import sys

sys.path.insert(0, "/opt/trn_rl_repo")

import numpy as np

import concourse.bass as bass
import concourse.mybir as mybir
import concourse.tile as tile
from concourse import bacc
from concourse.bass_utils import run_bass_kernel_spmd
from concourse.masks import make_identity

B, N, C, H = 8, 4096, 512, 8
HD = C // H                    # 64
SCALE = HD ** -0.5             # 1/8
MU = float(N)
NT = N // 128                  # 32 row tiles
CB = C // 128                  # 4 feature blocks
HP = H // 2                    # 4 head pairs
NG = 8                         # streaming groups
GW = N // NG                   # 512 rows/cols per group
TPG = NT // NG                 # 4 row tiles per group
F16 = mybir.dt.float16
F32 = mybir.dt.float32
AF = mybir.ActivationFunctionType

# Gram psum column ranges per row-block m (strict upper triangle)
GCOL = [(0, 512), (128, 512), (256, 512), (384, 512)]
# column offset of each m's accumulator inside the packed 3-bank psum
# tile: m1 (384 cols) and m3 (128 cols) share bank 1. m1 owns the bank's
# start (first write at t=0) and stop (last write at t=NT-1); m3 always
# runs with start=stop=False, relying on the bank's pending-zero bytes.
GOFF = [0, 512, 1024, 896]
GPW = 1536
# lower-triangle tiles needing a PE transpose
LOWT = [(1, 0), (2, 0), (2, 1), (3, 0), (3, 1), (3, 2)]


def build():
    nc = bacc.Bacc("TRN2", target_bir_lowering=False, debug=False, num_devices=8)
    x_d = [nc.declare_dram_parameter(f"x{s + 1}", [N, C], F16, isOutput=False)
           for s in range(2)]
    w_d = [nc.declare_dram_parameter(f"w{s + 1}", [C, 2 * C], F16, isOutput=False)
           for s in range(2)]
    o_d = [nc.declare_dram_parameter(f"ot{s + 1}", [C, N], F16, isOutput=True)
           for s in range(2)]

    with tile.TileContext(nc) as tc:
        with (
            tc.tile_pool(name="const", bufs=1) as constp,
            tc.tile_pool(name="wf", bufs=1) as wfp,
            tc.tile_pool(name="tts", bufs=1) as ttsp,
            tc.tile_pool(name="x", bufs=6) as xp,
            tc.tile_pool(name="xt", bufs=1) as xtp,
            tc.tile_pool(name="g", bufs=1) as gp_,
            tc.tile_pool(name="a", bufs=1) as ap_,
            tc.tile_pool(name="cx", bufs=1) as cxp,
            tc.tile_pool(name="osb", bufs=4) as osp,
            tc.tile_pool(name="ob1", bufs=HP) as ob1p,
            tc.tile_pool(name="ps_g", bufs=1, space="PSUM") as psg,
            tc.tile_pool(name="ps_t", bufs=1, space="PSUM") as pst,
            tc.tile_pool(name="ps_tx", bufs=1, space="PSUM") as pstx,
            tc.tile_pool(name="ps_o", bufs=3, space="PSUM") as pso,
        ):
            identf = constp.tile([128, 128], F32, tag="identf")
            make_identity(nc, identf[:])
            ident16 = constp.tile([128, 128], F16, tag="ident16")
            nc.scalar.copy(ident16[:], identf[:])
            muI = constp.tile([128, 128], F32, tag="muI")
            nc.gpsimd.memset(muI[:], 0.0)
            nc.gpsimd.affine_select(
                out=muI[:], in_=muI[:],
                compare_op=mybir.AluOpType.not_equal, fill=MU,
                base=0, pattern=[[-1, 128]], channel_multiplier=1,
            )

            xts = [xtp.tile([128, CB, N], F16, tag=f"xt{s}", name=f"xt{s}")
                   for s in range(2)]

            # ---- load order IS pipe order (one serialized DMA pipe):
            # x1 (first group split for an early start), weights, then x2
            # groups interleaved with the xT2 transpose-loads ----
            xcs = {}
            for g in range(NG):
                xc = xp.tile([128, TPG, C], F16, tag="xc", name=f"xc0_{g}")
                if g == 0:
                    for h in range(2):
                        nc.sync.dma_start(
                            out=xc[:, 2 * h:2 * h + 2, :],
                            in_=x_d[0][256 * h:256 * (h + 1), :].rearrange(
                                "(t p) c -> p t c", p=128))
                else:
                    nc.sync.dma_start(
                        out=xc[:],
                        in_=x_d[0][GW * g:GW * (g + 1), :].rearrange(
                            "(t p) c -> p t c", p=128))
                xcs[(0, g)] = xc

            wfs, ttss = [], []
            for s in range(2):
                wf = wfp.tile([128, CB, 2 * C], F16, tag=f"wf{s}")
                nc.sync.dma_start(
                    out=wf[:], in_=w_d[s][:, :].rearrange("(a p) m -> p a m", p=128))
                wfs.append(wf)
                ttss.append(ttsp.tile([128, HP, 128], F32, tag=f"tts{s}",
                                      name=f"tts{s}"))

            for g in range(NG):
                xc = xp.tile([128, TPG, C], F16, tag="xc", name=f"xc1_{g}")
                nc.sync.dma_start(
                    out=xc[:], in_=x_d[1][GW * g:GW * (g + 1), :].rearrange(
                        "(t p) c -> p t c", p=128))
                xcs[(1, g)] = xc

            eng = [nc.vector.tensor_copy, nc.scalar.copy]
            _ec = [0]

            def nxt_eng():
                # strict global alternation for out-phase evacuations so
                # interleaved streams never land two copies on one engine
                _ec[0] += 1
                return eng[_ec[0] % 2]

            def tt_weights(s):
                # exact TT = mu * Wv^T Wk, pair-packed [e(2h), d(2h)]
                wf = wfs[s]
                for hp in range(HP):
                    ttp = pso.tile([128, C], F32, tag="op", name=f"ttp{s}_{hp}")
                    for a in range(CB):
                        nc.tensor.matmul(
                            ttp[:, 0:128],
                            lhsT=wf[:, a, C + 128 * hp:C + 128 * (hp + 1)],
                            rhs=wf[:, a, 128 * hp:128 * (hp + 1)],
                            start=(a == 0), stop=(a == CB - 1))
                    nc.scalar.mul(ttss[s][:, hp, :], ttp[:, 0:128], MU)

            def gram_mm(s, t, gp):
                xc = xcs[(s, t // TPG)]
                tt_ = t % TPG
                order = [3, 1, 0, 2] if t == NT - 1 else [1, 3, 0, 2]
                for m in order:
                    lo, hi = GCOL[m]
                    nc.tensor.matmul(
                        gp[:, GOFF[m]:GOFF[m] + hi - lo],
                        lhsT=xc[:, tt_, 128 * m:128 * (m + 1)],
                        rhs=xc[:, tt_, lo:hi],
                        start=(t == 0 and m != 3),
                        stop=(t == NT - 1 and m != 3),
                        skip_group_check=(m == 3))

            tpx_cur = [None]

            def xpose(s, t):
                """PE-transpose x_s tile t into a paired psum tile
                (cb-major); evacuate once per 2 tiles."""
                xc = xcs[(s, t // TPG)]
                tt_ = t % TPG
                half = t % 2
                if half == 0:
                    tpx_cur[0] = pstx.tile([128, CB, 2, 128], F16, tag="tpx",
                                           name=f"tpx{s}_{t // 2}")
                tpx = tpx_cur[0]
                for cb in range(CB):
                    nc.tensor.transpose(
                        tpx[:, cb, half, :], xc[:, tt_, 128 * cb:128 * (cb + 1)],
                        ident16[:])
                if half == 1:
                    # out [p, a, 256] and in [p, a, 2, 128] flatten to the
                    # same per-partition element order
                    nxt_eng()(
                        xts[s][:, :, 128 * (t - 1):128 * (t + 1)], tpx[:])

            ctx_res = {}

            def g_stage1(s, gp):
                gsb = gp_.tile([128, CB, C], F16, tag="gsb", name=f"gsb{s}")
                for m in range(CB):
                    lo, hi = GCOL[m]
                    dg = GOFF[m] + 128 * m - lo
                    nc.vector.tensor_sub(
                        gsb[:, m, 128 * m:128 * (m + 1)], gp[:, dg:dg + 128],
                        muI[:])
                    if m < 3:
                        nxt_eng()(gsb[:, m, 128 * (m + 1):C],
                                  gp[:, dg + 128:GOFF[m] + hi - lo])
                return gsb

            def g_stage2(s, gsb):
                gtr = gp_.tile([128, len(LOWT), 128], F16, tag="gtr",
                               name=f"gtr{s}")
                for i, (a2, b2) in enumerate(LOWT):
                    tpg = pst.tile([128, CB, 128], F16, tag="tp",
                                   name=f"tpg{s}_{i}")
                    nc.tensor.transpose(
                        tpg[:, 0, :], gsb[:, b2, 128 * a2:128 * (a2 + 1)],
                        ident16[:])
                    nc.vector.tensor_copy(gtr[:, i, :], tpg[:, 0, :])
                low = {ab_: i for i, ab_ in enumerate(LOWT)}

                def g_tile(a2, b2):
                    if b2 >= a2:
                        return gsb[:, a2, 128 * b2:128 * (b2 + 1)]
                    return gtr[:, low[(a2, b2)], :]

                return g_tile

            def ctx_tail_gen(s, gp):
                """Staged G->A->ctp->softmax->cbd; yields between stages so
                the caller can interleave PE filler work. (Used for tail 1;
                tail 2 is written out per-head-pair below.)"""
                wf = wfs[s]
                gsb = g_stage1(s, gp)
                esb = cxp.tile([128, HP, 128], F32, tag="esb", name=f"esb{s}")
                ssum = cxp.tile([128, HP], F32, tag="ssum", name=f"ssum{s}")
                rsum = cxp.tile([128, HP], F32, tag="rsum", name=f"rsum{s}")
                comb = cxp.tile([128, HP, 128], F32, tag="comb", name=f"comb{s}")
                ctxts = cxp.tile([128, HP, 128], F16, tag="ctxts",
                                 name=f"ctxts{s}")
                nc.gpsimd.memset(ctxts[:], 0.0)
                yield
                g_tile = g_stage2(s, gsb)
                yield
                # A = Gc^T-tiles @ Wv (f16, free 512)
                ab = ap_.tile([128, CB, C], F16, tag="ab", name=f"ab{s}")
                for b2 in range(CB):
                    apx = pso.tile([128, C], F32, tag="op", name=f"apx{s}_{b2}")
                    for a2 in range(CB):
                        nc.tensor.matmul(
                            apx[:], lhsT=g_tile(a2, b2), rhs=wf[:, a2, C:2 * C],
                            start=(a2 == 0), stop=(a2 == CB - 1))
                    nxt_eng()(ab[:, b2, :], apx[:])
                    if b2 == 1:
                        yield
                yield
                # ctp (pair-packed) + TT, exp halves, per-hp reciprocal
                for hp in range(HP):
                    ctp = pso.tile([128, C], F32, tag="op", name=f"ctp{s}_{hp}")
                    sl = slice(128 * hp, 128 * (hp + 1))
                    for b2 in range(CB):
                        nc.tensor.matmul(
                            ctp[:, 0:128], lhsT=ab[:, b2, sl], rhs=wf[:, b2, sl],
                            start=(b2 == 0), stop=(b2 == CB - 1))
                    nc.vector.tensor_add(comb[:, hp, :], ctp[:, 0:128],
                                         ttss[s][:, hp, :])
                    nc.scalar.activation(
                        esb[0:64, hp, 0:64], comb[0:64, hp, 0:64], AF.Exp,
                        scale=SCALE, accum_out=ssum[0:64, hp:hp + 1])
                    nc.scalar.activation(
                        esb[64:128, hp, 64:128], comb[64:128, hp, 64:128], AF.Exp,
                        scale=SCALE, accum_out=ssum[64:128, hp:hp + 1])
                    nc.vector.reciprocal(rsum[:, hp:hp + 1], ssum[:, hp:hp + 1])
                    if hp % 2 == 1:
                        yield
                cbd = cxp.tile([128, HP, 128], F16, tag=f"cbd{s}")
                for hp in range(HP):
                    nc.vector.tensor_scalar_mul(
                        ctxts[0:64, hp, 0:64], esb[0:64, hp, 0:64],
                        rsum[0:64, hp:hp + 1])
                    nc.vector.tensor_scalar_mul(
                        ctxts[64:128, hp, 64:128], esb[64:128, hp, 64:128],
                        rsum[64:128, hp:hp + 1])
                    tpc = pst.tile([128, CB, 128], F16, tag="tp",
                                   name=f"tpc{s}_{hp}")
                    nc.tensor.transpose(tpc[:, 0, :], ctxts[:, hp, :],
                                        ident16[:])
                    nc.scalar.copy(cbd[:, hp, :], tpc[:, 0, :])
                ctx_res[s] = cbd

            # ================= phase 1: x1 gram + xpose =================
            # PE warm-up: dummy transposes bridge the DMA lead-in so the
            # p-state ramp is already progressing at the first gram matmul
            wtp = pst.tile([128, CB, 128], F16, tag="tp", name="wtp")
            for _ in range(14):
                nc.tensor.transpose(wtp[:, 0, :], ident16[:], ident16[:])
            nc.vector.tensor_copy(ident16[:], wtp[:, 0, :])

            gps1 = psg.tile([128, GPW], F32, tag="gp", name="gp_0")
            for t in range(NT):
                gram_mm(0, t, gps1)
                xpose(0, t)
                if t == 16:
                    tt_weights(0)
                if t == 20:
                    tt_weights(1)

            # ====== phase 2: x2 gram, interleaved with ctx_tail(1) ======
            gps2 = psg.tile([128, GPW], F32, tag="gp", name="gp_1")
            tail1 = ctx_tail_gen(0, gps1)
            next(tail1)          # emit G1 copies first (frees gram psum bank)
            t2 = 0               # phase-2 tile cursor

            def emit_tiles(k):
                nonlocal t2
                for _ in range(k):
                    if t2 >= NT:
                        return
                    gram_mm(1, t2, gps2)
                    xpose(1, t2)
                    t2 += 1

            for w in (3, 3, 2, 2, 2, 2, 2, 2, 2):
                emit_tiles(w)
                try:
                    next(tail1)
                except StopIteration:
                    break

            # o2^T staged per group; groups 0..5 woven into the remaining
            # gram tiles, the last 2 groups deferred into tail 2
            ob2s = {}

            def o2_mm(cb, g):
                if cb == 0:
                    ob2s[g] = osp.tile([128, CB, GW], F16, tag="ob",
                                       name=f"ob2_{g}")
                op = pso.tile([128, C], F32, tag="op", name=f"op2_{cb}_{g}")
                nc.tensor.matmul(
                    op[:], lhsT=ctx_res[0][:, cb, :],
                    rhs=xts[1][:, cb, GW * g:GW * (g + 1)],
                    start=True, stop=True)
                nxt_eng()(ob2s[g][:, cb, :], op[:])
                if cb == CB - 1:
                    nc.scalar.dma_start(
                        out=o_d[1][:, GW * g:GW * (g + 1)].rearrange(
                            "(a p) n -> p a n", p=128),
                        in_=ob2s[g][:])

            pend = [(cb, g) for g in range(NG - 4) for cb in range(CB)]
            pi = 0

            def emit_pend(k):
                nonlocal pi
                while k > 0 and pi < len(pend):
                    cb, g = pend[pi]
                    if t2 < TPG * (g + 1):
                        return
                    o2_mm(cb, g)
                    pi += 1
                    k -= 1

            while t2 < NT or pi < len(pend):
                emit_tiles(1)
                emit_pend(1)
                if t2 >= NT:
                    emit_pend(len(pend))

            defer_q = [(cb, g) for g in range(NG - 4, NG) for cb in range(CB)]
            di = 0

            def emit_defer(k):
                nonlocal di
                for _ in range(k):
                    if di >= len(defer_q):
                        return
                    o2_mm(*defer_q[di])
                    di += 1

            # -- software-pipelined tail 2: per-head-pair A->ctp->softmax
            # chains with the previous head-pair's o1 matmuls (and the
            # deferred o2 matmuls) woven between them --
            wf = wfs[1]
            gsb = g_stage1(1, gps2)
            emit_defer(6)
            g_tile1 = g_stage2(1, gsb)
            emit_defer(4)

            ab = ap_.tile([128, CB, C], F16, tag="ab", name="ab1")
            esb = cxp.tile([128, HP, 128], F32, tag="esb", name="esb1")
            ssum = cxp.tile([128, HP], F32, tag="ssum", name="ssum1")
            rsum = cxp.tile([128, HP], F32, tag="rsum", name="rsum1")
            comb = cxp.tile([128, HP, 128], F32, tag="comb", name="comb1")
            ctxts = cxp.tile([128, HP, 128], F16, tag="ctxts", name="ctxts1")
            nc.gpsimd.memset(ctxts[:], 0.0)
            cbd2 = cxp.tile([128, HP, 128], F16, tag="cbd1")
            ob1s = [ob1p.tile([128, N], F16, tag="ob1", name=f"ob1_{hp}")
                    for hp in range(HP)]

            def o1_hp(hp):
                # store in halves so the pipe drains alongside the matmuls
                for g in range(NG):
                    op = pso.tile([128, C], F32, tag="op", name=f"op3_{hp}_{g}")
                    nc.tensor.matmul(
                        op[:], lhsT=cbd2[:, hp, :],
                        rhs=xts[0][:, hp, GW * g:GW * (g + 1)],
                        start=True, stop=True)
                    nxt_eng()(ob1s[hp][:, GW * g:GW * (g + 1)], op[:])
                    if hp == HP - 1 and g % 2 == 1 and g < NG - 1:
                        # last head-pair: stream quarter-stores so the pipe
                        # drains alongside the final matmuls
                        q = g // 2
                        nc.scalar.dma_start(
                            out=o_d[0][128 * hp:128 * (hp + 1),
                                       1024 * q:1024 * (q + 1)],
                            in_=ob1s[hp][:, 1024 * q:1024 * (q + 1)])
                    elif hp < HP - 1 and g == NG // 2 - 1:
                        nc.scalar.dma_start(
                            out=o_d[0][128 * hp:128 * (hp + 1), 0:N // 2],
                            in_=ob1s[hp][:, 0:N // 2])
                last = N - 1024 if hp == HP - 1 else N // 2
                nc.scalar.dma_start(
                    out=o_d[0][128 * hp:128 * (hp + 1), last:N],
                    in_=ob1s[hp][:, last:N])

            for hp in range(HP):
                sl = slice(128 * hp, 128 * (hp + 1))
                for b2 in range(CB):
                    apx = pso.tile([128, C], F32, tag="op",
                                   name=f"a2_{hp}_{b2}")
                    for a2 in range(CB):
                        nc.tensor.matmul(
                            apx[:, 0:128], lhsT=g_tile1(a2, b2),
                            rhs=wf[:, a2, C + 128 * hp:C + 128 * (hp + 1)],
                            start=(a2 == 0), stop=(a2 == CB - 1))
                    nxt_eng()(ab[:, b2, sl], apx[:, 0:128])
                ctp = pso.tile([128, C], F32, tag="op", name=f"ctp1_{hp}")
                for b2 in range(CB):
                    nc.tensor.matmul(
                        ctp[:, 0:128], lhsT=ab[:, b2, sl], rhs=wf[:, b2, sl],
                        start=(b2 == 0), stop=(b2 == CB - 1))
                nc.vector.tensor_add(comb[:, hp, :], ctp[:, 0:128],
                                     ttss[1][:, hp, :])
                nc.scalar.activation(
                    esb[0:64, hp, 0:64], comb[0:64, hp, 0:64], AF.Exp,
                    scale=SCALE, accum_out=ssum[0:64, hp:hp + 1])
                nc.scalar.activation(
                    esb[64:128, hp, 64:128], comb[64:128, hp, 64:128], AF.Exp,
                    scale=SCALE, accum_out=ssum[64:128, hp:hp + 1])
                nc.vector.reciprocal(rsum[:, hp:hp + 1], ssum[:, hp:hp + 1])
                nc.vector.tensor_scalar_mul(
                    ctxts[0:64, hp, 0:64], esb[0:64, hp, 0:64],
                    rsum[0:64, hp:hp + 1])
                nc.vector.tensor_scalar_mul(
                    ctxts[64:128, hp, 64:128], esb[64:128, hp, 64:128],
                    rsum[64:128, hp:hp + 1])
                tpc = pst.tile([128, CB, 128], F16, tag="tp", name=f"tpc1_{hp}")
                nc.tensor.transpose(tpc[:, 0, :], ctxts[:, hp, :], ident16[:])
                nc.scalar.copy(cbd2[:, hp, :], tpc[:, 0, :])
                # previous head-pair's o1 matmuls run while this chain's
                # softmax settles on V/S
                if hp >= 1:
                    o1_hp(hp - 1)
                emit_defer(5)
            o1_hp(HP - 1)
            emit_defer(len(defer_q))
    nc.compile()
    return nc


_NC = None


def make_in_maps(inputs):
    x1 = np.asarray(inputs["x1"])
    x2 = np.asarray(inputs["x2"])
    w1 = np.ascontiguousarray(np.asarray(inputs["W_kv1"]), dtype=np.float16)
    w2 = np.ascontiguousarray(np.asarray(inputs["W_kv2"]), dtype=np.float16)
    in_maps = []
    for b in range(B):
        in_maps.append({
            "x1": np.ascontiguousarray(x1[b], dtype=np.float16),
            "x2": np.ascontiguousarray(x2[b], dtype=np.float16),
            "w1": w1, "w2": w2,
        })
    return in_maps


def kernel(x1, x2, W_kv1, W_kv2):
    global _NC
    if _NC is None:
        _NC = build()
    in_maps = make_in_maps(
        {"x1": x1, "x2": x2, "W_kv1": W_kv1, "W_kv2": W_kv2})
    res = run_bass_kernel_spmd(_NC, in_maps, core_ids=list(range(B)))
    o1 = np.stack([res.results[b]["ot1"].astype(np.float32).T
                   for b in range(B)])
    o2 = np.stack([res.results[b]["ot2"].astype(np.float32).T
                   for b in range(B)])
    return o1, o2


# revision 54
# speedup vs baseline: 1.0090x; 1.0090x over previous
"""Trainium2 Bass kernel for nn_CrossAttention_249108103802.

8 cores data-parallel over B=8; per core (batch b):
  G_s   = x_s^T x_s            (Gram, fp16 operands, fp32 psum, upper tri)
  A_s   = (G_s - mu I) Wv_s    (fp16)
  ctp_s = A_s^T Wk_s + mu Wv_s^T Wk_s   (fp16 pair-packed + fp32 TT)
  ctx_s = softmax_d(scale * ctp_s)      (per-head 64x64)
  o2^T  = blockdiag(ctx1) stationary @ xT2   fused into x2 streaming
  o1^T  = same with ctx2 / resident xT1

Everything is tuned against the cost model: fp16 operands (1 col/cycle,
half the DMA), PE kept gap-free (p-state halves the clock for 3us after
any idle), a single serialized DMA pipe (issue order == transfer order).
xT1 is built with PE transposes during phase 1 (PE has slack there);
xT2 arrives via DMA-transpose loads (pipe has slack, and phase 2's
copy engines are freed to evacuate the fused o2 matmuls). The softmax
tails are interleaved with matmul filler (phase-2 gram tiles for
tail 1; deferred o2 + pipelined o1 head-pair streams for tail 2).
Outputs are written as o^T [C, N] fp16 and transposed back on host.
"""
import sys

sys.path.insert(0, "/opt/trn_rl_repo")

import numpy as np

import concourse.bass as bass
import concourse.mybir as mybir
import concourse.tile as tile
from concourse import bacc
from concourse.bass_utils import run_bass_kernel_spmd
from concourse.masks import make_identity

B, N, C, H = 8, 4096, 512, 8
HD = C // H                    # 64
SCALE = HD ** -0.5             # 1/8
MU = float(N)
NT = N // 128                  # 32 row tiles
CB = C // 128                  # 4 feature blocks
HP = H // 2                    # 4 head pairs
NG = 8                         # streaming groups
GW = N // NG                   # 512 rows/cols per group
TPG = NT // NG                 # 4 row tiles per group
F16 = mybir.dt.float16
F32 = mybir.dt.float32
AF = mybir.ActivationFunctionType

# Gram psum column ranges per row-block m (strict upper triangle)
GCOL = [(0, 512), (128, 512), (256, 512), (384, 512)]
# column offset of each m's accumulator inside the packed 3-bank psum
# tile: m1 (384 cols) and m3 (128 cols) share bank 1. m1 owns the bank's
# start (first write at t=0) and stop (last write at t=NT-1); m3 always
# runs with start=stop=False, relying on the bank's pending-zero bytes.
GOFF = [0, 512, 1024, 896]
GPW = 1536
# lower-triangle tiles needing a PE transpose
LOWT = [(1, 0), (2, 0), (2, 1), (3, 0), (3, 1), (3, 2)]


def build():
    nc = bacc.Bacc("TRN2", target_bir_lowering=False, debug=False, num_devices=8)
    x_d = [nc.declare_dram_parameter(f"x{s + 1}", [N, C], F16, isOutput=False)
           for s in range(2)]
    w_d = [nc.declare_dram_parameter(f"w{s + 1}", [C, 2 * C], F16, isOutput=False)
           for s in range(2)]
    o_d = [nc.declare_dram_parameter(f"ot{s + 1}", [C, N], F16, isOutput=True)
           for s in range(2)]

    with tile.TileContext(nc) as tc:
        with (
            tc.tile_pool(name="const", bufs=1) as constp,
            tc.tile_pool(name="wf", bufs=1) as wfp,
            tc.tile_pool(name="tts", bufs=1) as ttsp,
            tc.tile_pool(name="x", bufs=6) as xp,
            tc.tile_pool(name="xt", bufs=1) as xtp,
            tc.tile_pool(name="g", bufs=1) as gp_,
            tc.tile_pool(name="a", bufs=1) as ap_,
            tc.tile_pool(name="cx", bufs=1) as cxp,
            tc.tile_pool(name="osb", bufs=4) as osp,
            tc.tile_pool(name="ob1", bufs=HP) as ob1p,
            tc.tile_pool(name="ps_g", bufs=1, space="PSUM") as psg,
            tc.tile_pool(name="ps_t", bufs=1, space="PSUM") as pst,
            tc.tile_pool(name="ps_tx", bufs=1, space="PSUM") as pstx,
            tc.tile_pool(name="ps_o", bufs=3, space="PSUM") as pso,
        ):
            identf = constp.tile([128, 128], F32, tag="identf")
            make_identity(nc, identf[:])
            ident16 = constp.tile([128, 128], F16, tag="ident16")
            nc.scalar.copy(ident16[:], identf[:])
            muI = constp.tile([128, 128], F32, tag="muI")
            nc.gpsimd.memset(muI[:], 0.0)
            nc.gpsimd.affine_select(
                out=muI[:], in_=muI[:],
                compare_op=mybir.AluOpType.not_equal, fill=MU,
                base=0, pattern=[[-1, 128]], channel_multiplier=1,
            )

            xts = [xtp.tile([128, CB, N], F16, tag=f"xt{s}", name=f"xt{s}")
                   for s in range(2)]

            # ---- load order IS pipe order (one serialized DMA pipe):
            # x1 (first group split for an early start), weights, then x2
            # groups interleaved with the xT2 transpose-loads ----
            xcs = {}
            for g in range(NG):
                xc = xp.tile([128, TPG, C], F16, tag="xc", name=f"xc0_{g}")
                if g == 0:
                    for h in range(2):
                        nc.sync.dma_start(
                            out=xc[:, 2 * h:2 * h + 2, :],
                            in_=x_d[0][256 * h:256 * (h + 1), :].rearrange(
                                "(t p) c -> p t c", p=128))
                else:
                    nc.sync.dma_start(
                        out=xc[:],
                        in_=x_d[0][GW * g:GW * (g + 1), :].rearrange(
                            "(t p) c -> p t c", p=128))
                xcs[(0, g)] = xc

            wfs, ttss = [], []
            for s in range(2):
                wf = wfp.tile([128, CB, 2 * C], F16, tag=f"wf{s}")
                nc.sync.dma_start(
                    out=wf[:], in_=w_d[s][:, :].rearrange("(a p) m -> p a m", p=128))
                wfs.append(wf)
                ttss.append(ttsp.tile([128, HP, 128], F32, tag=f"tts{s}",
                                      name=f"tts{s}"))

            for g in range(NG):
                xc = xp.tile([128, TPG, C], F16, tag="xc", name=f"xc1_{g}")
                nc.sync.dma_start(
                    out=xc[:], in_=x_d[1][GW * g:GW * (g + 1), :].rearrange(
                        "(t p) c -> p t c", p=128))
                xcs[(1, g)] = xc

            eng = [nc.vector.tensor_copy, nc.scalar.copy]
            _ec = [0]

            def nxt_eng():
                # strict global alternation for out-phase evacuations so
                # interleaved streams never land two copies on one engine
                _ec[0] += 1
                return eng[_ec[0] % 2]

            def tt_weights(s):
                # exact TT = mu * Wv^T Wk, pair-packed [e(2h), d(2h)]
                wf = wfs[s]
                for hp in range(HP):
                    ttp = pso.tile([128, C], F32, tag="op", name=f"ttp{s}_{hp}")
                    for a in range(CB):
                        nc.tensor.matmul(
                            ttp[:, 0:128],
                            lhsT=wf[:, a, C + 128 * hp:C + 128 * (hp + 1)],
                            rhs=wf[:, a, 128 * hp:128 * (hp + 1)],
                            start=(a == 0), stop=(a == CB - 1))
                    nc.scalar.mul(ttss[s][:, hp, :], ttp[:, 0:128], MU)

            def gram_mm(s, t, gp):
                xc = xcs[(s, t // TPG)]
                tt_ = t % TPG
                order = [3, 1, 0, 2] if t == NT - 1 else [1, 3, 0, 2]
                for m in order:
                    lo, hi = GCOL[m]
                    nc.tensor.matmul(
                        gp[:, GOFF[m]:GOFF[m] + hi - lo],
                        lhsT=xc[:, tt_, 128 * m:128 * (m + 1)],
                        rhs=xc[:, tt_, lo:hi],
                        start=(t == 0 and m != 3),
                        stop=(t == NT - 1 and m != 3),
                        skip_group_check=(m == 3))

            tpx_cur = [None]

            def xpose(s, t):
                """PE-transpose x_s tile t into a paired psum tile
                (cb-major); evacuate once per 2 tiles."""
                xc = xcs[(s, t // TPG)]
                tt_ = t % TPG
                half = t % 2
                if half == 0:
                    tpx_cur[0] = pstx.tile([128, CB, 2, 128], F16, tag="tpx",
                                           name=f"tpx{s}_{t // 2}")
                tpx = tpx_cur[0]
                for cb in range(CB):
                    nc.tensor.transpose(
                        tpx[:, cb, half, :], xc[:, tt_, 128 * cb:128 * (cb + 1)],
                        ident16[:])
                if half == 1:
                    # out [p, a, 256] and in [p, a, 2, 128] flatten to the
                    # same per-partition element order
                    eng[(t // 2) % 2](
                        xts[s][:, :, 128 * (t - 1):128 * (t + 1)], tpx[:])

            ctx_res = {}

            def g_stage1(s, gp):
                gsb = gp_.tile([128, CB, C], F16, tag="gsb", name=f"gsb{s}")
                for m in range(CB):
                    lo, hi = GCOL[m]
                    dg = GOFF[m] + 128 * m - lo
                    nc.vector.tensor_sub(
                        gsb[:, m, 128 * m:128 * (m + 1)], gp[:, dg:dg + 128],
                        muI[:])
                    if m < 3:
                        eng[m % 2](gsb[:, m, 128 * (m + 1):C],
                                   gp[:, dg + 128:GOFF[m] + hi - lo])
                return gsb

            def g_stage2(s, gsb):
                gtr = gp_.tile([128, len(LOWT), 128], F16, tag="gtr",
                               name=f"gtr{s}")
                for i, (a2, b2) in enumerate(LOWT):
                    tpg = pst.tile([128, CB, 128], F16, tag="tp",
                                   name=f"tpg{s}_{i}")
                    nc.tensor.transpose(
                        tpg[:, 0, :], gsb[:, b2, 128 * a2:128 * (a2 + 1)],
                        ident16[:])
                    nc.vector.tensor_copy(gtr[:, i, :], tpg[:, 0, :])
                low = {ab_: i for i, ab_ in enumerate(LOWT)}

                def g_tile(a2, b2):
                    if b2 >= a2:
                        return gsb[:, a2, 128 * b2:128 * (b2 + 1)]
                    return gtr[:, low[(a2, b2)], :]

                return g_tile

            def ctx_tail_gen(s, gp):
                """Staged G->A->ctp->softmax->cbd; yields between stages so
                the caller can interleave PE filler work. (Used for tail 1;
                tail 2 is written out per-head-pair below.)"""
                wf = wfs[s]
                gsb = g_stage1(s, gp)
                esb = cxp.tile([128, HP, 128], F32, tag="esb", name=f"esb{s}")
                ssum = cxp.tile([128, HP], F32, tag="ssum", name=f"ssum{s}")
                rsum = cxp.tile([128, HP], F32, tag="rsum", name=f"rsum{s}")
                comb = cxp.tile([128, HP, 128], F32, tag="comb", name=f"comb{s}")
                ctxts = cxp.tile([128, HP, 128], F16, tag="ctxts",
                                 name=f"ctxts{s}")
                nc.gpsimd.memset(ctxts[:], 0.0)
                yield
                g_tile = g_stage2(s, gsb)
                yield
                # A = Gc^T-tiles @ Wv (f16, free 512)
                ab = ap_.tile([128, CB, C], F16, tag="ab", name=f"ab{s}")
                for b2 in range(CB):
                    apx = pso.tile([128, C], F32, tag="op", name=f"apx{s}_{b2}")
                    for a2 in range(CB):
                        nc.tensor.matmul(
                            apx[:], lhsT=g_tile(a2, b2), rhs=wf[:, a2, C:2 * C],
                            start=(a2 == 0), stop=(a2 == CB - 1))
                    eng[b2 % 2](ab[:, b2, :], apx[:])
                    if b2 == 1:
                        yield
                yield
                # ctp (pair-packed) + TT, exp halves, per-hp reciprocal
                for hp in range(HP):
                    ctp = pso.tile([128, C], F32, tag="op", name=f"ctp{s}_{hp}")
                    sl = slice(128 * hp, 128 * (hp + 1))
                    for b2 in range(CB):
                        nc.tensor.matmul(
                            ctp[:, 0:128], lhsT=ab[:, b2, sl], rhs=wf[:, b2, sl],
                            start=(b2 == 0), stop=(b2 == CB - 1))
                    nc.vector.tensor_add(comb[:, hp, :], ctp[:, 0:128],
                                         ttss[s][:, hp, :])
                    nc.scalar.activation(
                        esb[0:64, hp, 0:64], comb[0:64, hp, 0:64], AF.Exp,
                        scale=SCALE, accum_out=ssum[0:64, hp:hp + 1])
                    nc.scalar.activation(
                        esb[64:128, hp, 64:128], comb[64:128, hp, 64:128], AF.Exp,
                        scale=SCALE, accum_out=ssum[64:128, hp:hp + 1])
                    nc.vector.reciprocal(rsum[:, hp:hp + 1], ssum[:, hp:hp + 1])
                    if hp % 2 == 1:
                        yield
                cbd = cxp.tile([128, HP, 128], F16, tag=f"cbd{s}")
                for hp in range(HP):
                    nc.vector.tensor_scalar_mul(
                        ctxts[0:64, hp, 0:64], esb[0:64, hp, 0:64],
                        rsum[0:64, hp:hp + 1])
                    nc.vector.tensor_scalar_mul(
                        ctxts[64:128, hp, 64:128], esb[64:128, hp, 64:128],
                        rsum[64:128, hp:hp + 1])
                    tpc = pst.tile([128, CB, 128], F16, tag="tp",
                                   name=f"tpc{s}_{hp}")
                    nc.tensor.transpose(tpc[:, 0, :], ctxts[:, hp, :],
                                        ident16[:])
                    nc.scalar.copy(cbd[:, hp, :], tpc[:, 0, :])
                ctx_res[s] = cbd

            # ================= phase 1: x1 gram + xpose =================
            # PE warm-up: dummy transposes bridge the DMA lead-in so the
            # p-state ramp is already progressing at the first gram matmul
            wtp = pst.tile([128, CB, 128], F16, tag="tp", name="wtp")
            for _ in range(14):
                nc.tensor.transpose(wtp[:, 0, :], ident16[:], ident16[:])
            nc.vector.tensor_copy(ident16[:], wtp[:, 0, :])

            gps1 = psg.tile([128, GPW], F32, tag="gp", name="gp_0")
            for t in range(NT):
                gram_mm(0, t, gps1)
                xpose(0, t)
                if t == 16:
                    tt_weights(0)
                if t == 20:
                    tt_weights(1)

            # ====== phase 2: x2 gram, interleaved with ctx_tail(1) ======
            gps2 = psg.tile([128, GPW], F32, tag="gp", name="gp_1")
            tail1 = ctx_tail_gen(0, gps1)
            next(tail1)          # emit G1 copies first (frees gram psum bank)
            t2 = 0               # phase-2 tile cursor

            def emit_tiles(k):
                nonlocal t2
                for _ in range(k):
                    if t2 >= NT:
                        return
                    gram_mm(1, t2, gps2)
                    xpose(1, t2)
                    t2 += 1

            for w in (3, 3, 2, 2, 2, 2, 2, 2, 2):
                emit_tiles(w)
                try:
                    next(tail1)
                except StopIteration:
                    break

            # o2^T staged per group; groups 0..5 woven into the remaining
            # gram tiles, the last 2 groups deferred into tail 2
            ob2s = {}

            def o2_mm(cb, g):
                if cb == 0:
                    ob2s[g] = osp.tile([128, CB, GW], F16, tag="ob",
                                       name=f"ob2_{g}")
                op = pso.tile([128, C], F32, tag="op", name=f"op2_{cb}_{g}")
                nc.tensor.matmul(
                    op[:], lhsT=ctx_res[0][:, cb, :],
                    rhs=xts[1][:, cb, GW * g:GW * (g + 1)],
                    start=True, stop=True)
                nxt_eng()(ob2s[g][:, cb, :], op[:])
                if cb == CB - 1:
                    nc.scalar.dma_start(
                        out=o_d[1][:, GW * g:GW * (g + 1)].rearrange(
                            "(a p) n -> p a n", p=128),
                        in_=ob2s[g][:])

            pend = [(cb, g) for g in range(NG - 4) for cb in range(CB)]
            pi = 0

            def emit_pend(k):
                nonlocal pi
                while k > 0 and pi < len(pend):
                    cb, g = pend[pi]
                    if t2 < TPG * (g + 1):
                        return
                    o2_mm(cb, g)
                    pi += 1
                    k -= 1

            while t2 < NT or pi < len(pend):
                emit_tiles(1)
                emit_pend(1)
                if t2 >= NT:
                    emit_pend(len(pend))

            defer_q = [(cb, g) for g in range(NG - 4, NG) for cb in range(CB)]
            di = 0

            def emit_defer(k):
                nonlocal di
                for _ in range(k):
                    if di >= len(defer_q):
                        return
                    o2_mm(*defer_q[di])
                    di += 1

            # -- software-pipelined tail 2: per-head-pair A->ctp->softmax
            # chains with the previous head-pair's o1 matmuls (and the
            # deferred o2 matmuls) woven between them --
            wf = wfs[1]
            gsb = g_stage1(1, gps2)
            emit_defer(6)
            g_tile1 = g_stage2(1, gsb)
            emit_defer(4)

            ab = ap_.tile([128, CB, C], F16, tag="ab", name="ab1")
            esb = cxp.tile([128, HP, 128], F32, tag="esb", name="esb1")
            ssum = cxp.tile([128, HP], F32, tag="ssum", name="ssum1")
            rsum = cxp.tile([128, HP], F32, tag="rsum", name="rsum1")
            comb = cxp.tile([128, HP, 128], F32, tag="comb", name="comb1")
            ctxts = cxp.tile([128, HP, 128], F16, tag="ctxts", name="ctxts1")
            nc.gpsimd.memset(ctxts[:], 0.0)
            cbd2 = cxp.tile([128, HP, 128], F16, tag="cbd1")
            ob1s = [ob1p.tile([128, N], F16, tag="ob1", name=f"ob1_{hp}")
                    for hp in range(HP)]

            def o1_hp(hp):
                # store in halves so the pipe drains alongside the matmuls
                for g in range(NG):
                    op = pso.tile([128, C], F32, tag="op", name=f"op3_{hp}_{g}")
                    nc.tensor.matmul(
                        op[:], lhsT=cbd2[:, hp, :],
                        rhs=xts[0][:, hp, GW * g:GW * (g + 1)],
                        start=True, stop=True)
                    nxt_eng()(ob1s[hp][:, GW * g:GW * (g + 1)], op[:])
                    if hp == HP - 1 and g % 2 == 1 and g < NG - 1:
                        # last head-pair: stream quarter-stores so the pipe
                        # drains alongside the final matmuls
                        q = g // 2
                        nc.scalar.dma_start(
                            out=o_d[0][128 * hp:128 * (hp + 1),
                                       1024 * q:1024 * (q + 1)],
                            in_=ob1s[hp][:, 1024 * q:1024 * (q + 1)])
                    elif hp < HP - 1 and g == NG // 2 - 1:
                        nc.scalar.dma_start(
                            out=o_d[0][128 * hp:128 * (hp + 1), 0:N // 2],
                            in_=ob1s[hp][:, 0:N // 2])
                last = N - 1024 if hp == HP - 1 else N // 2
                nc.scalar.dma_start(
                    out=o_d[0][128 * hp:128 * (hp + 1), last:N],
                    in_=ob1s[hp][:, last:N])

            for hp in range(HP):
                sl = slice(128 * hp, 128 * (hp + 1))
                for b2 in range(CB):
                    apx = pso.tile([128, C], F32, tag="op",
                                   name=f"a2_{hp}_{b2}")
                    for a2 in range(CB):
                        nc.tensor.matmul(
                            apx[:, 0:128], lhsT=g_tile1(a2, b2),
                            rhs=wf[:, a2, C + 128 * hp:C + 128 * (hp + 1)],
                            start=(a2 == 0), stop=(a2 == CB - 1))
                    eng[b2 % 2](ab[:, b2, sl], apx[:, 0:128])
                ctp = pso.tile([128, C], F32, tag="op", name=f"ctp1_{hp}")
                for b2 in range(CB):
                    nc.tensor.matmul(
                        ctp[:, 0:128], lhsT=ab[:, b2, sl], rhs=wf[:, b2, sl],
                        start=(b2 == 0), stop=(b2 == CB - 1))
                nc.vector.tensor_add(comb[:, hp, :], ctp[:, 0:128],
                                     ttss[1][:, hp, :])
                nc.scalar.activation(
                    esb[0:64, hp, 0:64], comb[0:64, hp, 0:64], AF.Exp,
                    scale=SCALE, accum_out=ssum[0:64, hp:hp + 1])
                nc.scalar.activation(
                    esb[64:128, hp, 64:128], comb[64:128, hp, 64:128], AF.Exp,
                    scale=SCALE, accum_out=ssum[64:128, hp:hp + 1])
                nc.vector.reciprocal(rsum[:, hp:hp + 1], ssum[:, hp:hp + 1])
                nc.vector.tensor_scalar_mul(
                    ctxts[0:64, hp, 0:64], esb[0:64, hp, 0:64],
                    rsum[0:64, hp:hp + 1])
                nc.vector.tensor_scalar_mul(
                    ctxts[64:128, hp, 64:128], esb[64:128, hp, 64:128],
                    rsum[64:128, hp:hp + 1])
                tpc = pst.tile([128, CB, 128], F16, tag="tp", name=f"tpc1_{hp}")
                nc.tensor.transpose(tpc[:, 0, :], ctxts[:, hp, :], ident16[:])
                nc.scalar.copy(cbd2[:, hp, :], tpc[:, 0, :])
                # previous head-pair's o1 matmuls run while this chain's
                # softmax settles on V/S
                if hp >= 1:
                    o1_hp(hp - 1)
                emit_defer(5)
            o1_hp(HP - 1)
            emit_defer(len(defer_q))
    nc.compile()
    return nc


_NC = None


def make_in_maps(inputs):
    x1 = np.asarray(inputs["x1"])
    x2 = np.asarray(inputs["x2"])
    w1 = np.ascontiguousarray(np.asarray(inputs["W_kv1"]), dtype=np.float16)
    w2 = np.ascontiguousarray(np.asarray(inputs["W_kv2"]), dtype=np.float16)
    in_maps = []
    for b in range(B):
        in_maps.append({
            "x1": np.ascontiguousarray(x1[b], dtype=np.float16),
            "x2": np.ascontiguousarray(x2[b], dtype=np.float16),
            "w1": w1, "w2": w2,
        })
    return in_maps


def kernel(x1, x2, W_kv1, W_kv2):
    global _NC
    if _NC is None:
        _NC = build()
    in_maps = make_in_maps(
        {"x1": x1, "x2": x2, "W_kv1": W_kv1, "W_kv2": W_kv2})
    res = run_bass_kernel_spmd(_NC, in_maps, core_ids=list(range(B)))
    o1 = np.stack([res.results[b]["ot1"].astype(np.float32).T
                   for b in range(B)])
    o2 = np.stack([res.results[b]["ot2"].astype(np.float32).T
                   for b in range(B)])
    return o1, o2
